# revision 2
# baseline (speedup 1.0000x reference)
"""BiLSTM-CRF (L=2048, V=50000, E=H=512, K=11) on 8 Trainium2 NeuronCores.

Self-contained harness entry point: kernel(**inputs) -> (score, path).

Sharding strategy (SPMD-uniform program, per-core data differs):
- Even cores run the forward LSTM, odd cores the backward LSTM (the backward
  cores receive the reversed sentence and the backward weights in the same
  input slots).
- Per direction: embedding rows are gathered on-device with dma_gather (two
  passes to cover vocab 50000 > int16 range, merged with a mask), x is
  transposed on the TensorEngine, xg = x @ Wih.T + b is precomputed as a big
  bf16 matmul, then the sequential LSTM recurrence runs as bf16 matvecs on the
  TensorEngine with sigmoid-only activations (tanh(x) = 2*sigmoid(2x)-1, the
  inner 2x folded into the g-gate weights on the host).
- An AllGather over core pairs exchanges the hidden states; every core then
  redundantly computes feats = [hf;hb] @ W_tag.T + b_tag and runs a forward
  and a backward max-plus Viterbi DP entirely on the VectorEngine (32x32
  stream transposes to flip the max reduction onto the free axis). The path
  is extracted vectorized over time as argmax_k(alpha_t[k] + beta_t[k]) with
  first-index tie-breaking; the score is max_k(alpha_0[k] + beta_0[k]).
"""

import numpy as np
import ml_dtypes

import concourse.bass as bass
import concourse.mybir as mybir
import concourse.tile as tile
from concourse import bacc
from concourse.masks import make_identity

dt = mybir.dt
Alu = mybir.AluOpType
Act = mybir.ActivationFunctionType
Ax = mybir.AxisListType

V, E, H, K = 50000, 512, 512, 11
G = 4 * H
KP = 32
START, STOP = 9, 10
NEG = -10000.0
VS = 32768  # int16-safe vocab split for dma_gather
P = 128
L = 2048
N_CORES = 8
bf16 = ml_dtypes.bfloat16


# --------------------------------------------------------------------------
# host-side input prep
# --------------------------------------------------------------------------

def _prep_core_inputs(inputs, reverse):
    sent = np.asarray(inputs["sentence"]).astype(np.int64)
    assert sent.shape == (L,)
    if reverse:
        sent = sent[::-1].copy()

    idx_lo = np.where(sent < VS, sent, 0).astype(np.int16)
    idx_hi = np.where(sent >= VS, sent - VS, 0).astype(np.int16)

    def wrap16(a):
        # [channels=16, L//16], replicated to all 128 partitions (the 8
        # GPSIMD cores each read their own 16-partition copy on HW)
        return np.tile(a.reshape(L // 16, 16).T, (8, 1)).copy()

    hi_mask = (sent >= VS).astype(np.float32).reshape(L // P, P).T.copy()

    sfx = "b" if reverse else "f"
    Wih = np.asarray(inputs[f"Wih_{sfx}"], np.float32)
    Whh = np.asarray(inputs[f"Whh_{sfx}"], np.float32)
    bih = np.asarray(inputs[f"bih_{sfx}"], np.float32)
    bhh = np.asarray(inputs[f"bhh_{sfx}"], np.float32)
    scale = np.ones(G, np.float32)
    scale[2 * H:3 * H] = 2.0  # tanh(g) = 2*sigmoid(2g)-1

    wih_t = (Wih.T * scale[None, :]).astype(bf16)
    whh_t = (Whh.T * scale[None, :]).astype(bf16)
    bias_g = ((bih + bhh) * scale).reshape(16, P).T.copy().astype(np.float32)

    d = 1 if reverse else 0
    h0 = np.asarray(inputs["h0"], np.float32)[d].reshape(4, P).T.copy().astype(bf16)
    c0 = np.asarray(inputs["c0"], np.float32)[d].reshape(4, P).T.copy()

    W_tag = np.asarray(inputs["W_tag"], np.float32)
    wtag_t = np.zeros((2 * H, 16), bf16)
    wtag_t[:, :K] = W_tag.T.astype(bf16)
    btag = np.full((KP, 1), NEG, np.float32)
    btag[:K, 0] = np.asarray(inputs["b_tag"], np.float32)

    T = np.asarray(inputs["transitions"], np.float32)
    trans_a = np.full((KP, KP), NEG, np.float32)
    trans_a[:K, :K] = T.T
    trans_b = np.full((KP, KP), NEG, np.float32)
    trans_b[:K, :K] = T
    trans_b0 = trans_b.copy()
    trans_b0[:, START] = NEG  # terminal mask: last tag may not be START
    init_a = np.full((KP, 1), NEG, np.float32)
    init_a[START, 0] = 0.0
    init_b = np.full((KP, 1), NEG, np.float32)
    init_b[STOP, 0] = 0.0
    iota32 = np.tile(np.arange(KP, dtype=np.float32), (KP, 1))

    return {
        "embed": np.asarray(inputs["embed"], np.float32),
        "idx_lo": wrap16(idx_lo),
        "idx_hi": wrap16(idx_hi),
        "hi_mask": hi_mask,
        "wih_t": wih_t,
        "whh_t": whh_t,
        "bias_g": bias_g,
        "h0_bf": h0,
        "c0_f": c0,
        "wtag_t": wtag_t,
        "btag": btag,
        "trans_a": trans_a,
        "trans_b": trans_b,
        "trans_b0": trans_b0,
        "init_a": init_a,
        "init_b": init_b,
        "iota32": iota32,
    }


def make_in_maps(inputs, n_cores=N_CORES):
    lo = _prep_core_inputs(inputs, reverse=False)
    hi = _prep_core_inputs(inputs, reverse=True)
    return [dict(lo if c % 2 == 0 else hi) for c in range(n_cores)]


def decode_outputs(res0):
    score = np.float32(res0["score_out"][0, 0])
    path = np.rint(res0["path_out"].T.reshape(-1)).astype(np.int32)
    return score, path


# --------------------------------------------------------------------------
# device program (identical on every core)
# --------------------------------------------------------------------------

def core_kernel(tc, outs, ins, n_cores=N_CORES):
    nc = tc.nc
    NTILE = min(512, L)
    NT = L // NTILE
    TB = L // P
    NB = L // KP
    f32 = dt.float32
    b16 = dt.bfloat16

    embed = ins["embed"]
    replica_groups = [[i, i + 1] for i in range(0, n_cores, 2)]

    with (
        tc.tile_pool(name="persist", bufs=1) as PP,
        tc.tile_pool(name="work", bufs=3) as W,
        tc.tile_pool(name="psum", bufs=2, space="PSUM") as PS,
        tc.tile_pool(name="dram", bufs=1, space="DRAM") as DR,
    ):
        h_all = PP.tile([P, 4, L + 1], b16)
        c_sb = PP.tile([P, 4], f32)
        bias_sb = PP.tile([P, 16], f32)
        feats = PP.tile([KP, L], f32)
        u_fw = PP.tile([KP, L + 1], f32)
        u_bw = PP.tile([KP, L + 1], f32)
        trans_a = PP.tile([KP, KP], f32)
        trans_b = PP.tile([KP, KP], f32)
        trans_b0 = PP.tile([KP, KP], f32)
        iota_sb = PP.tile([KP, KP], f32)
        btag_sb = PP.tile([KP, 1], f32)
        wtag_sb = PP.tile([P, 8, 16], b16)
        ident = PP.tile([P, P], f32)

        nc.sync.dma_start(bias_sb[:], ins["bias_g"][:])
        nc.sync.dma_start(h_all[:, :, 0], ins["h0_bf"][:])
        nc.sync.dma_start(c_sb[:], ins["c0_f"][:])
        nc.sync.dma_start(trans_a[:], ins["trans_a"][:])
        nc.sync.dma_start(trans_b[:], ins["trans_b"][:])
        nc.sync.dma_start(trans_b0[:], ins["trans_b0"][:])
        nc.sync.dma_start(iota_sb[:], ins["iota32"][:])
        nc.sync.dma_start(btag_sb[:], ins["btag"][:])
        nc.sync.dma_start(u_fw[:, 0:1], ins["init_a"][:])
        nc.sync.dma_start(u_bw[:, L:L + 1], ins["init_b"][:])
        nc.sync.dma_start(
            wtag_sb[:], ins["wtag_t"].rearrange("(c p) j -> p c j", p=P))
        make_identity(nc, ident[:])

        with tc.tile_pool(name="xg", bufs=1) as PXG:
            xg_all = PXG.tile([P, 16, L], b16)

            with tc.tile_pool(name="xT", bufs=1) as PXT:
                xT = PXT.tile([P, 4, L], b16)

                with tc.tile_pool(name="xbuf", bufs=1) as PXB:
                    xb1 = PXB.tile([P, TB, E], f32)
                    xb2 = PXB.tile([P, TB, E], f32)
                    idx1 = PXB.tile([P, L // 16], dt.int16)
                    idx2 = PXB.tile([P, L // 16], dt.int16)
                    msk = PXB.tile([P, TB], f32)
                    nc.sync.dma_start(idx1[:], ins["idx_lo"][:])
                    nc.sync.dma_start(idx2[:], ins["idx_hi"][:])
                    nc.sync.dma_start(msk[:], ins["hi_mask"][:])
                    nc.gpsimd.dma_gather(
                        xb1[:], embed[0:VS, :], idx1[:],
                        num_idxs=L, num_idxs_reg=L, elem_size=E)
                    nc.gpsimd.dma_gather(
                        xb2[:], embed[VS:V, :], idx2[:],
                        num_idxs=L, num_idxs_reg=L, elem_size=E)
                    lo_m = PXB.tile([P, TB], f32)
                    nc.vector.tensor_scalar(
                        out=lo_m[:], in0=msk[:], scalar1=-1.0, scalar2=1.0,
                        op0=Alu.mult, op1=Alu.add)
                    for tb in range(TB):
                        nc.vector.tensor_scalar(
                            out=xb1[:, tb, :], in0=xb1[:, tb, :],
                            scalar1=lo_m[:, tb:tb + 1], scalar2=None,
                            op0=Alu.mult)
                        nc.vector.scalar_tensor_tensor(
                            xb1[:, tb, :], in0=xb2[:, tb, :],
                            scalar=msk[:, tb:tb + 1], in1=xb1[:, tb, :],
                            op0=Alu.mult, op1=Alu.add)
                    for tb in range(TB):
                        for ec in range(4):
                            pst = PS.tile([P, P], f32, tag="ps_tr")
                            nc.tensor.transpose(
                                pst[:], xb1[:, tb, ec * P:(ec + 1) * P],
                                ident[:])
                            nc.vector.tensor_copy(
                                out=xT[:, ec, tb * P:(tb + 1) * P],
                                in_=pst[:])

                with tc.tile_pool(name="wih", bufs=1) as PWIH:
                    wih_sb = PWIH.tile([P, 4, G], b16)
                    nc.sync.dma_start(
                        wih_sb[:],
                        ins["wih_t"].rearrange("(k p) g -> p k g", p=P))
                    for m in range(16):
                        for nt in range(NT):
                            psx = PS.tile([P, NTILE], f32, tag="ps_xg")
                            for k in range(4):
                                nc.tensor.matmul(
                                    psx[:],
                                    wih_sb[:, k, m * P:(m + 1) * P],
                                    xT[:, k, nt * NTILE:(nt + 1) * NTILE],
                                    start=(k == 0), stop=(k == 3))
                            nc.vector.tensor_scalar(
                                out=xg_all[:, m, nt * NTILE:(nt + 1) * NTILE],
                                in0=psx[:], scalar1=bias_sb[:, m:m + 1],
                                scalar2=None, op0=Alu.add)

            with tc.tile_pool(name="whh", bufs=1) as PWHH:
                whh_sb = PWHH.tile([P, 4, G], b16)
                nc.sync.dma_start(
                    whh_sb[:],
                    ins["whh_t"].rearrange("(k p) g -> p k g", p=P))
                for t in range(L):
                    psg = PS.tile([P, 16], f32, tag="ps_g")
                    for m in range(16):
                        for k in range(4):
                            nc.tensor.matmul(
                                psg[:, m:m + 1],
                                whh_sb[:, k, m * P:(m + 1) * P],
                                h_all[:, k, t:t + 1],
                                start=(k == 0), stop=(k == 3))
                    g_sb = W.tile([P, 16], f32, tag="g")
                    nc.vector.tensor_tensor(
                        g_sb[:], psg[:], xg_all[:, :, t], Alu.add)
                    sg = W.tile([P, 16], f32, tag="sg")
                    nc.scalar.activation(sg[:], g_sb[:], Act.Sigmoid)
                    t1 = W.tile([P, 4], f32, tag="t1")
                    nc.vector.tensor_tensor(
                        t1[:], sg[:, 8:12], sg[:, 0:4], Alu.mult)
                    t2 = W.tile([P, 4], f32, tag="t2")
                    nc.vector.tensor_tensor(
                        t2[:], c_sb[:], sg[:, 4:8], Alu.mult)
                    t3 = W.tile([P, 4], f32, tag="t3")
                    nc.vector.scalar_tensor_tensor(
                        t3[:], in0=t1[:], scalar=2.0, in1=sg[:, 0:4],
                        op0=Alu.mult, op1=Alu.subtract)
                    nc.vector.tensor_tensor(c_sb[:], t2[:], t3[:], Alu.add)
                    sc = W.tile([P, 4], f32, tag="sc")
                    nc.scalar.activation(
                        sc[:], c_sb[:], Act.Sigmoid, scale=2.0)
                    t5 = W.tile([P, 4], f32, tag="t5")
                    nc.vector.tensor_tensor(
                        t5[:], sc[:], sg[:, 12:16], Alu.mult)
                    nc.vector.scalar_tensor_tensor(
                        h_all[:, :, t + 1], in0=t5[:], scalar=2.0,
                        in1=sg[:, 12:16], op0=Alu.mult, op1=Alu.subtract)

        with tc.tile_pool(name="post", bufs=1) as PO:
            bounce_in = DR.tile([P, 4 * L], b16)
            bounce_out = DR.tile([2 * P, 4 * L], b16)
            nc.sync.dma_start(bounce_in[:], h_all[:, :, 1:L + 1])
            nc.gpsimd.collective_compute(
                "AllGather", Alu.bypass,
                replica_groups=replica_groups,
                ins=[bounce_in[:].opt()],
                outs=[bounce_out[:].opt()],
            )
            hf_sb = PO.tile([P, 4, L], b16)
            hb_rv = PO.tile([P, 4, L], b16)
            hb_sb = PO.tile([P, 4, L], b16)
            nc.sync.dma_start(
                hf_sb[:],
                bounce_out[0:P, :].rearrange("p (k t) -> p k t", k=4))
            nc.sync.dma_start(
                hb_rv[:],
                bounce_out[P:2 * P, :].rearrange("p (k t) -> p k t", k=4))
            nc.vector.tensor_copy(out=hb_sb[:], in_=hb_rv[:, :, ::-1])

            nc.vector.memset(feats[:], NEG)
            for nt in range(NT):
                psf = PS.tile([16, NTILE], f32, tag="ps_f")
                for c8 in range(8):
                    rhs = (hf_sb if c8 < 4 else hb_sb)[
                        :, c8 % 4, nt * NTILE:(nt + 1) * NTILE]
                    nc.tensor.matmul(
                        psf[:], wtag_sb[:, c8, :], rhs,
                        start=(c8 == 0), stop=(c8 == 7))
                nc.vector.tensor_scalar(
                    out=feats[0:16, nt * NTILE:(nt + 1) * NTILE],
                    in0=psf[:], scalar1=btag_sb[0:16, 0:1], scalar2=None,
                    op0=Alu.add)

            for t in range(L):
                vx = W.tile([KP, KP], f32, tag="vx")
                nc.vector.tensor_scalar(
                    out=vx[:], in0=trans_a[:], scalar1=u_fw[:, t:t + 1],
                    scalar2=None, op0=Alu.add)
                vy = W.tile([KP, KP], f32, tag="vy")
                nc.vector.transpose(vy[:], vx[:])
                vm = W.tile([KP, 1], f32, tag="vm")
                nc.vector.tensor_reduce(vm[:], vy[:], Ax.X, Alu.max)
                nc.vector.tensor_tensor(
                    u_fw[:, t + 1:t + 2], vm[:], feats[:, t:t + 1], Alu.add)

            for s in range(L):
                t = L - 1 - s
                tb_sb = trans_b0 if s == 0 else trans_b
                vx = W.tile([KP, KP], f32, tag="vx")
                nc.vector.tensor_scalar(
                    out=vx[:], in0=tb_sb[:], scalar1=u_bw[:, t + 1:t + 2],
                    scalar2=None, op0=Alu.add)
                vy = W.tile([KP, KP], f32, tag="vy")
                nc.vector.transpose(vy[:], vx[:])
                vm = W.tile([KP, 1], f32, tag="vm")
                nc.vector.tensor_reduce(vm[:], vy[:], Ax.X, Alu.max)
                nc.vector.tensor_tensor(
                    u_bw[:, t:t + 1], vm[:], feats[:, t:t + 1], Alu.add)

            S = PO.tile([KP, L], f32)
            nc.vector.tensor_tensor(S[:], u_fw[:, 1:L + 1], u_bw[:, 0:L],
                                    Alu.add)
            nc.vector.tensor_tensor(S[:], S[:], feats[:], Alu.subtract)
            St = PO.tile([KP, NB, KP], f32)
            for b in range(NB):
                nc.vector.transpose(St[:, b, :], S[:, b * KP:(b + 1) * KP])
            mrow = PO.tile([KP, NB], f32)
            nc.vector.tensor_reduce(mrow[:], St[:], Ax.X, Alu.max)
            score_sb = W.tile([1, 1], f32, tag="sco")
            nc.vector.tensor_reduce(score_sb[:], St[0:1, 0:1, :], Ax.X,
                                    Alu.max)
            nc.sync.dma_start(outs["score_out"][:], score_sb[:])
            ohlt = PO.tile([KP, NB, KP], f32)
            nc.vector.tensor_tensor(
                ohlt[:], St[:],
                mrow[:, :, None].to_broadcast([KP, NB, KP]), Alu.is_lt)
            masked = PO.tile([KP, NB, KP], f32)
            nc.vector.scalar_tensor_tensor(
                masked[:], in0=ohlt[:], scalar=1000.0,
                in1=iota_sb[:, None, :].to_broadcast([KP, NB, KP]),
                op0=Alu.mult, op1=Alu.add)
            tags = PO.tile([KP, NB], f32)
            nc.vector.tensor_reduce(tags[:], masked[:], Ax.X, Alu.min)
            nc.sync.dma_start(outs["path_out"][:], tags[:])


IN_SPECS = {
    "embed": ([V, E], dt.float32),
    "idx_lo": ([P, L // 16], dt.int16),
    "idx_hi": ([P, L // 16], dt.int16),
    "hi_mask": ([P, L // P], dt.float32),
    "wih_t": ([E, G], dt.bfloat16),
    "whh_t": ([H, G], dt.bfloat16),
    "bias_g": ([P, 16], dt.float32),
    "h0_bf": ([P, 4], dt.bfloat16),
    "c0_f": ([P, 4], dt.float32),
    "wtag_t": ([2 * H, 16], dt.bfloat16),
    "btag": ([KP, 1], dt.float32),
    "trans_a": ([KP, KP], dt.float32),
    "trans_b": ([KP, KP], dt.float32),
    "trans_b0": ([KP, KP], dt.float32),
    "init_a": ([KP, 1], dt.float32),
    "init_b": ([KP, 1], dt.float32),
    "iota32": ([KP, KP], dt.float32),
}


def build(n_cores=N_CORES):
    nc = bacc.Bacc("TRN2", target_bir_lowering=False, debug=False,
                   num_devices=n_cores)
    ins = {name: nc.dram_tensor(name, shape, d, kind="ExternalInput").ap()
           for name, (shape, d) in IN_SPECS.items()}
    outs = {
        "score_out": nc.dram_tensor("score_out", [1, 1], dt.float32,
                                    kind="ExternalOutput").ap(),
        "path_out": nc.dram_tensor("path_out", [KP, L // KP], dt.float32,
                                   kind="ExternalOutput").ap(),
    }
    with tile.TileContext(nc) as tc:
        core_kernel(tc, outs, ins, n_cores)
    nc.compile()
    return nc


_CACHE = {}


def kernel(**inputs):
    if "nc" not in _CACHE:
        _CACHE["nc"] = build(N_CORES)
    nc = _CACHE["nc"]
    in_maps = make_in_maps(inputs, N_CORES)
    from concourse.bass_utils import run_bass_kernel_spmd
    res = run_bass_kernel_spmd(nc, in_maps, core_ids=list(range(N_CORES)))
    score, path = decode_outputs(res.results[0])
    return score, path
